# revision 14
# baseline (speedup 1.0000x reference)
"""Trainium2 Bass kernel for a conditional GRU decoder.

Model (per reference):
  h0 = [z, x_cond] @ W_lh.T + b_lh
  x0 = 0
  for t in 0..127:
      hn = GRUCell(x_t, h_t);  logits_t = hn @ W_out.T + b_out;  x_{t+1} = hn
  out = (B, 128, 64)

Because x_{t+1} == h_{t+1} for t >= 1, the two GRU matmuls fuse into one
(B,R) @ (R,4R) matmul with W_fused rows [Wi_r+Wh_r; Wi_z+Wh_z; Wi_n; Wh_n].

Sharding: data-parallel over batch, B=2048 -> 8 cores x 256. All weights
replicated. On-chip layout is transposed (feature dim on partitions, batch on
the free dim) so gate biases are per-partition scalars and the recurrent
matmuls keep weights stationary:
    gatesT[4R, b] = W_fused @ hT   via  matmul(out, lhsT=W_fused.T, rhs=hT)

Per-step engine assignment (vs the original all-DVE formulation):
  - b_r/b_z are pre-added into the gr/gz PSUM banks by a rank-2 K=2 matmul
    (lhsT = bias chunks, rhs = chunk-selector), so each sigmoid is a single
    full-width FD=512 ACT op with no per-chunk bias.
  - pre_n = t2 + gn is accumulated on the PE (identity matmul into gn's
    PSUM bank) instead of a DVE tensor_tensor.
  - update: p1m = (u-1)*nt (one DVE STT), h' = p2 - p1m, with p2 = u*h on
    GpSimd. This removes the v=1-u op entirely.
  - all DVE operands are f16 SBUF (2x perf mode) except PSUM-sourced ops.
  - logits bias-add runs on DVE (tensor_scalar with per-partition bias AP).
Matmul operands are float16; elementwise precision mixes f16/fp32.
"""

import numpy as np

import concourse.bass as bass
import concourse.tile as tile
from concourse import bacc, mybir
from concourse.bass_utils import run_bass_kernel_spmd

F32 = mybir.dt.float32
F16 = mybir.dt.float16
ACT = mybir.ActivationFunctionType
ALU = mybir.AluOpType

B = 2048
HID = 256
COND = 128
NCH = 64
MAXLEN = 128
R = 256
NCORES = 8
BC = B // NCORES  # 256 per-core batch
KT = R // 128     # 2 k-tiles over R
ZC = HID + COND   # 384
ZKT = ZC // 128   # 3 k-tiles over hid+cond


def _build():
    nc = bacc.Bacc("TRN2", target_bir_lowering=False, debug=False)

    # ---- DRAM I/O (per-core shapes) ----
    d_zct = nc.dram_tensor("zct", [ZC, BC], F16, kind="ExternalInput")
    d_wf = nc.dram_tensor("wft", [R, 4 * R], F16, kind="ExternalInput")
    d_whh = nc.dram_tensor("whht", [R, 3 * R], F16, kind="ExternalInput")
    d_wlh = nc.dram_tensor("wlht", [ZC, R], F16, kind="ExternalInput")
    d_wout = nc.dram_tensor("woutt", [R, NCH], F16, kind="ExternalInput")
    # bias columns: 0,1=b_r  2,3=b_z  4,5=b_in  6,7=b_hn  8,9=b_lh
    d_bias = nc.dram_tensor("biases", [128, 10], F32, kind="ExternalInput")
    d_bout = nc.dram_tensor("bout", [NCH, 1], F32, kind="ExternalInput")
    # rank-1 bias prefill operands: cols [b_r c0 | b_r c1 | b_z c0 | b_z c1]
    d_bpre = nc.dram_tensor("bpre", [1, 512], F16, kind="ExternalInput")
    d_bsel = nc.dram_tensor("bsel", [1, BC], F16, kind="ExternalInput")
    d_ident = nc.dram_tensor("ident", [128, 128], F16, kind="ExternalInput")
    d_out = nc.dram_tensor("out", [MAXLEN, NCH, BC], F32, kind="ExternalOutput")
    d_dbg = nc.dram_tensor("dbg", [128, BC], F32, kind="ExternalOutput")

    with tile.TileContext(nc) as tc:
        with (
            tc.tile_pool(name="const", bufs=1) as const,
            tc.tile_pool(name="state", bufs=1) as state,
            tc.tile_pool(name="ew", bufs=2) as ew,
            tc.tile_pool(name="pg", bufs=1, space="PSUM") as pg,
            tc.tile_pool(name="pg2", bufs=2, space="PSUM") as pg2,
            tc.tile_pool(name="pl", bufs=1, space="PSUM") as pl,
            tc.tile_pool(name="pk", bufs=1, space="PSUM") as pk,
        ):
            # ---- load constants ----
            wf = const.tile([128, KT, 4 * R], F16)
            nc.sync.dma_start(wf, d_wf[:].rearrange("(k p) m -> p k m", p=128))
            whh = const.tile([128, KT, 3 * R], F16)
            nc.sync.dma_start(whh, d_whh[:].rearrange("(k p) m -> p k m", p=128))
            wlh = const.tile([128, ZKT, R], F16)
            nc.sync.dma_start(wlh, d_wlh[:].rearrange("(k p) m -> p k m", p=128))
            wout = const.tile([128, KT, NCH], F16)
            nc.sync.dma_start(wout, d_wout[:].rearrange("(k p) m -> p k m", p=128))
            zct = const.tile([128, ZKT, BC], F16)
            nc.sync.dma_start(zct, d_zct[:].rearrange("(k p) m -> p k m", p=128))
            bia = const.tile([128, 10], F32)
            nc.sync.dma_start(bia, d_bias[:])
            boutc = const.tile([NCH, 1], F32)
            nc.sync.dma_start(boutc, d_bout[:])
            bpre = const.tile([1, 512], F16)
            nc.sync.dma_start(bpre, d_bpre[:])
            bsel = const.tile([1, BC], F16)
            nc.sync.dma_start(bsel, d_bsel[:])
            ident = const.tile([128, 128], F16)
            nc.sync.dma_start(ident, d_ident[:])

            def bcol(i):
                return bia[:, i : i + 1]

            def ck(ap, m):  # chunk m of a flat [128, 2*BC] tile
                return ap[:, bass.ds(m * BC, BC)]

            h = state.tile([128, KT * BC], F16)  # hT flat, chunk c=rows c*128..
            # PE keepalive: accumulating dummy matmuls on const inputs keep
            # the HAM clock-gate at full rate through the per-step EW gap.
            ka = pk.tile([128, BC], F32)
            ka_started = [False]

            def keepalive_on(rhs):
                nc.tensor.matmul(ka, wf[:, 0, 0:128], rhs,
                                 start=(not ka_started[0]), stop=False,
                                 skip_group_check=True)
                ka_started[0] = True

            # ---- h0 = W_lh @ zcT + b_lh ----
            ph = pg.tile([128, KT * BC], F32, tag="gr")
            for m in range(KT):
                for k in range(ZKT):
                    nc.tensor.matmul(ck(ph, m), wlh[:, k, bass.ts(m, 128)],
                                     zct[:, k, :], start=(k == 0),
                                     stop=(k == ZKT - 1))
            for m in range(KT):
                nc.scalar.activation(ck(h, m), ck(ph, m), ACT.Identity,
                                     bias=bcol(8 + m))

            # ---- per-step bodies ----
            # pending logits matmuls from the previous step are emitted in the
            # middle of this step's gate burst so they never gate the chain.
            pending = []

            def flush_logits():
                for fn in pending:
                    fn()
                pending.clear()

            def emit_step(t, first):
                if first:
                    # x=0: gates come from W_hh only (pytorch order r,z,n)
                    w, offs = whh, {"r": 0, "z": R, "hn": 2 * R}
                else:
                    w, offs = wf, {"r": 0, "z": R, "in": 2 * R, "hn": 3 * R}

                gr = pg.tile([128, KT * BC], F32, tag="gr")
                gz = pg.tile([128, KT * BC], F32, tag="gz")
                ghn = pg2.tile([128, KT * BC], F32, tag="ghn")
                gn = None if first else pg2.tile([128, KT * BC], F32, tag="gn")

                # PSUM accumulation flags are bank-granular: exactly one
                # start=True (first MM touching the bank; clears the whole
                # bank's has_written) and one stop=True (last MM).
                def prefill(dst, col):  # per-chunk rank-1 bias write
                    for m in range(KT):
                        nc.tensor.matmul(ck(dst, m),
                                         bpre[:, bass.ds(col + m * 128, 128)],
                                         bsel, start=(m == 0), stop=False)

                def gate_mms(dst, name, prefilled=False, open_end=False):
                    for m in range(KT):
                        for k in range(KT):
                            nc.tensor.matmul(
                                ck(dst, m),
                                w[:, k, bass.ds(offs[name] + m * 128, 128)],
                                ck(h, k),
                                start=(m == 0 and k == 0) and not prefilled,
                                stop=(m == KT - 1 and k == KT - 1)
                                and not open_end)

                # -- PE burst: r gate, z gate, prev logits, hn gate, n gate --
                prefill(gr, 0)
                gate_mms(gr, "r", prefilled=True)
                prefill(gz, 256)
                gate_mms(gz, "z", prefilled=True)
                flush_logits()
                gate_mms(ghn, "hn")
                if not first:
                    gate_mms(gn, "in", open_end=True)

                # r = sigmoid(gr) full-width (bias pre-added in PSUM)
                r = ew.tile([128, KT * BC], F16, tag="r")
                nc.scalar.activation(r, gr[:, :], ACT.Sigmoid)
                u = ew.tile([128, KT * BC], F16, tag="u")
                nc.scalar.activation(u, gz[:, :], ACT.Sigmoid)
                # p2 = u*h on GpSimd (off the serial chain)
                p2 = ew.tile([128, KT * BC], F16, tag="p2")
                nc.gpsimd.tensor_mul(p2, u, h)
                # t2 = (ghn + b_hn) * r
                t2 = ew.tile([128, KT * BC], F16, tag="t2")
                for m in range(KT):
                    nc.vector.scalar_tensor_tensor(
                        ck(t2, m), ck(ghn, m), bcol(6 + m), ck(r, m),
                        op0=ALU.add, op1=ALU.mult)
                # pre_n = gn + t2 accumulated on the PE into gn's PSUM bank
                if not first:
                    for m in range(KT):
                        nc.tensor.matmul(ck(gn, m), ident, ck(t2, m),
                                         start=False, stop=(m == KT - 1))
                keepalive_on(ck(t2, 0))
                # n = tanh(pre_n + b_in)
                nt = ew.tile([128, KT * BC], F16, tag="nt")
                src_n = t2 if first else gn
                for m in range(KT):
                    nc.scalar.activation(ck(nt, m), ck(src_n, m), ACT.Tanh,
                                         bias=bcol(4 + m))
                keepalive_on(ck(nt, 0))
                for _ in range(2):
                    keepalive_on(zct[:, 0, :])
                # h' = p2 - (u-1)*nt, chunk-staggered so next-step k=0 MMs
                # start as early as possible
                p1m = ew.tile([128, KT * BC], F16, tag="p1m")
                for m in range(KT):
                    nc.vector.scalar_tensor_tensor(
                        ck(p1m, m), ck(u, m), 1.0, ck(nt, m),
                        op0=ALU.subtract, op1=ALU.mult)
                    nc.vector.tensor_tensor(ck(h, m), ck(p2, m), ck(p1m, m),
                                            op=ALU.subtract)
                # logits_t = W_out @ h' + b_out -- deferred into next burst
                lp = pl.tile([NCH, BC], F32, tag="lp")

                def do_logits(lp=lp, t=t):
                    for k in range(KT):
                        nc.tensor.matmul(lp, wout[:, k, :], ck(h, k),
                                         start=(k == 0), stop=(k == KT - 1))
                    ls = ew.tile([NCH, BC], F32, tag="ls")
                    nc.vector.tensor_scalar(ls, lp, boutc[:, 0:1], None,
                                            op0=ALU.add)
                    nc.sync.dma_start(d_out[t], ls)
                pending.append(do_logits)

            emit_step(0, first=True)
            for t in range(1, MAXLEN):
                emit_step(t, first=False)
            flush_logits()
            kcopy = ew.tile([128, BC], F32, tag="kcopy")
            nc.scalar.activation(kcopy, ka, ACT.Identity, bias=0.0)
            nc.sync.dma_start(d_dbg[:], kcopy)

    nc.compile()
    return nc


_CACHE = {}
_LAST_IN_MAPS = None


def kernel(z, x_cond, W_lh, b_lh, W_ih, W_hh, b_ih, b_hh, W_out, b_out):
    z = np.asarray(z, np.float32)
    x_cond = np.asarray(x_cond, np.float32)
    W_lh = np.asarray(W_lh, np.float32)
    b_lh = np.asarray(b_lh, np.float32)
    W_ih = np.asarray(W_ih, np.float32)
    W_hh = np.asarray(W_hh, np.float32)
    b_ih = np.asarray(b_ih, np.float32)
    b_hh = np.asarray(b_hh, np.float32)
    W_out = np.asarray(W_out, np.float32)
    b_out = np.asarray(b_out, np.float32)

    # fused recurrent weight: rows [Wi_r+Wh_r; Wi_z+Wh_z; Wi_n; Wh_n]
    Wf = np.concatenate(
        [W_ih[:R] + W_hh[:R], W_ih[R : 2 * R] + W_hh[R : 2 * R],
         W_ih[2 * R :], W_hh[2 * R :]], axis=0)
    b_r = b_ih[:R] + b_hh[:R]
    b_z = b_ih[R : 2 * R] + b_hh[R : 2 * R]
    b_in = b_ih[2 * R :]
    b_hn = b_hh[2 * R :]

    def pcols(v):  # (R,) -> (128, KT) per-partition columns
        return np.ascontiguousarray(v.reshape(KT, 128).T)

    biases = np.ascontiguousarray(
        np.concatenate([pcols(b_r), pcols(b_z), pcols(b_in), pcols(b_hn),
                        pcols(b_lh)], axis=1))  # (128, 10)

    f16 = np.float16
    wft = np.ascontiguousarray(Wf.T, dtype=f16)            # (R, 4R)
    whht = np.ascontiguousarray(W_hh.T, dtype=f16)         # (R, 3R)
    wlht = np.ascontiguousarray(W_lh.T, dtype=f16)         # (ZC, R)
    woutt = np.ascontiguousarray(W_out.T, dtype=f16)       # (R, NCH)
    boutr = np.ascontiguousarray(b_out.reshape(NCH, 1))
    zct_full = np.concatenate([z, x_cond], axis=1).T.astype(f16)  # (ZC, B)

    bpre = np.ascontiguousarray(
        np.concatenate([b_r, b_z]).reshape(1, 512), dtype=f16)
    bsel = np.ones((1, BC), dtype=f16)
    ident = np.eye(128, dtype=f16)

    if "nc" not in _CACHE:
        _CACHE["nc"] = _build()
    nc = _CACHE["nc"]

    in_maps = []
    for c in range(NCORES):
        in_maps.append({
            "zct": np.ascontiguousarray(zct_full[:, c * BC : (c + 1) * BC]),
            "wft": wft,
            "whht": whht,
            "wlht": wlht,
            "woutt": woutt,
            "biases": biases,
            "bout": boutr,
            "bpre": bpre,
            "bsel": bsel,
            "ident": ident,
        })

    global _LAST_IN_MAPS
    _LAST_IN_MAPS = in_maps
    res = run_bass_kernel_spmd(nc, in_maps, core_ids=list(range(NCORES)))
    # per-core out: (MAXLEN, NCH, BC) -> (BC, MAXLEN, NCH)
    parts = [np.asarray(res.results[c]["out"]).transpose(2, 0, 1)
             for c in range(NCORES)]
    return np.ascontiguousarray(np.concatenate(parts, axis=0), dtype=np.float32)
